# revision 38
# baseline (speedup 1.0000x reference)
"""Distributed multi-head attention kernel for one TRN2 chip (8 NeuronCores).

Problem: x[2,2048,1024] -> qkv proj (W_qkv[3072,1024], b_qkv) -> 16-head
attention (d_key=64) -> out proj (W_o[1024,1024], b_o).

Sharding: head tensor-parallel, 2 heads per core.  Everything on-device is
computed in transposed orientation so no transposes are ever needed.

Logits run as ONE fp8 DoubleRow matmul per (key-tile, head) via a packed
hi/lo split (0.5 cycles/row -- half the bf16 cost):

    k_pack_h [128, 4096]: partitions 0:64 = fp8_hi(64*k_h), 64:128 = the
        fp8 residual lo part (h1 stores [lo; hi] -- partition order within
        the contraction is irrelevant).  The DR matmul's lhsT j-dim is a
        stride-0 broadcast of this single copy.
    q_pack_h [128, 2, 4096]: j=0 = [q_hi; q_hi], j=1 = [q_lo; q_lo]
        (both partition halves identical; the duplicate halves are written
        by the gpsimd engine, the natural halves by the DVE during the q
        projection drain, with b_q folded in).
    logitsT[keys, q] = sum_p,j lhsT[p,j,m] rhs[p,j,n]
                     = k_hi.q_hi + k_hi.q_lo + k_lo.q_hi + k_lo.q_lo
    i.e. the FULL hi/lo product at 256 cycles per 512-column tile instead
    of 512.  Packed-split accuracy is ~1e-3 -- better than bf16.

  v [128pos x 32tile x 130] natural orientation, with "ones" columns at 64
  and 129 so the ctx matmul's lhsT = [v_h | ones] produces the softmax
  denominator for free on psum partition 64.  exp runs on the Activation
  engine ([128,1024] tiles, scale folds the 64^2 weight scale + 1/8);
  the ctx accumulation (bf16) is deferred a full superiteration behind the
  exps so the PE never stalls on the exp chain.

  Projection matmuls run as fp8e4m3 DoubleRow pairs; weights pre-scaled by
  64 and split hi+lo on the host; three cross terms recover ~bf16 accuracy
  at ~1/3 the PE cost.  b_k dropped (softmax shift invariance), b_v folded
  into b_o' = b_o + W_o @ b_v (pre-broadcast to [128,1024] on the host),
  b_q added by the DVE during the q pack drain.

  Output rows are owned interleaved; per-row-group AllToAlls fire right
  after their two attention superiterations; the first three collectives
  hide under later attention, and all eight output-projection groups run
  inside the final collective's window, topped up with throwaway warm
  matmuls that keep the PE's p-state at maximum.

Matmul/compute dtype bf16/fp8-split (f32 PSUM accumulation); rel-err gate
is 2e-2.
"""

import sys

sys.path.insert(0, "/opt/trn_rl_repo")

import numpy as np
import ml_dtypes

import concourse.bass as bass
import concourse.tile as tile
from concourse import bacc, mybir
from concourse.bass_utils import run_bass_kernel_spmd

BF16 = mybir.dt.bfloat16
F32 = mybir.dt.float32
FP8 = mybir.dt.float8e4
NPBF16 = ml_dtypes.bfloat16
NPFP8 = ml_dtypes.float8_e4m3
DR = mybir.MatmulPerfMode.DoubleRow
SUB = mybir.AluOpType.subtract

D = 1024  # d_model
T = 2048  # seq len
B = 2  # batch
P = B * T  # 4096 total positions
H = 16  # total heads
DK = 64  # head dim
NCORES = 8
HL = H // NCORES  # 2 heads per core

WSCALE = 64.0  # host-side weight scale keeping fp8 operands in normal range
EXP_SCALE = 0.125 / (WSCALE * WSCALE)  # softmax exp scale incl. 64^2 logits


def build_graph(reps=1):
    nc = bacc.Bacc(
        "TRN2", target_bir_lowering=False, debug=False, num_devices=NCORES
    )

    # --- per-core external inputs ---
    x_hi = nc.declare_dram_parameter("x_hi", [D, P], FP8, isOutput=False)
    x_lo = nc.declare_dram_parameter("x_lo", [D, P], FP8, isOutput=False)
    wq_hi = nc.declare_dram_parameter("wq_hi", [D, 128], FP8, isOutput=False)
    wq_lo = nc.declare_dram_parameter("wq_lo", [D, 128], FP8, isOutput=False)
    wk_hi = nc.declare_dram_parameter("wk_hi", [D, 128], FP8, isOutput=False)
    wk_lo = nc.declare_dram_parameter("wk_lo", [D, 128], FP8, isOutput=False)
    wv_hi = nc.declare_dram_parameter("wv_hi", [D, 130], FP8, isOutput=False)
    wv_lo = nc.declare_dram_parameter("wv_lo", [D, 130], FP8, isOutput=False)
    bq = nc.declare_dram_parameter("bq", [128, 1], F32, isOutput=False)
    woT = nc.declare_dram_parameter("woT", [D, D], BF16, isOutput=False)
    bo = nc.declare_dram_parameter("bo", [128, D], BF16, isOutput=False)
    out = nc.declare_dram_parameter("out", [P // NCORES, D], F32, isOutput=True)

    with tile.TileContext(nc) as tc:
        with (
            tc.tile_pool(name="const", bufs=1) as const_pool,
            tc.tile_pool(name="xw", bufs=1) as xw_pool,
            tc.tile_pool(name="qkv", bufs=1) as qkv_pool,
            tc.tile_pool(name="et", bufs=16) as et_pool,
            tc.tile_pool(name="norm", bufs=2) as norm_pool,
            tc.tile_pool(name="ctxn", bufs=4) as ctxn_pool,
            tc.tile_pool(name="ow", bufs=1) as ow_pool,
            tc.tile_pool(name="obuf", bufs=2) as obuf_pool,
            tc.tile_pool(name="ps_mm", bufs=2, space="PSUM") as ps_mm,
            tc.tile_pool(name="ps_log", bufs=2, space="PSUM") as ps_log,
            tc.tile_pool(name="ps_ctx", bufs=2, space="PSUM") as ps_ctx,
            tc.tile_pool(name="dram", bufs=1, space="DRAM") as dram_pool,
        ):
            # tiny constants on the gpsimd queue so sync/scalar HWDGE start
            # on the weights/x immediately
            bq_sb = const_pool.tile([128, 1], F32)
            bo_full = const_pool.tile([128, D], BF16)
            nc.gpsimd.dma_start(out=bq_sb, in_=bq[:, :])
            ones65_sb = const_pool.tile([65, 128], BF16)
            nc.vector.memset(ones65_sb, 1.0)
            warm_src = const_pool.tile([128, 512], BF16)
            nc.vector.memset(warm_src, 0.5)

            env = dict(locals())
            env.pop("env", None)
            for rep in range(reps):
                emit_body(nc, tc, env)

    nc.compile()
    return nc


def emit_body(nc, tc, env):
    """Emit one full forward pass.

    Engine instruction streams are in-order, so the emission schedule is a
    hand software-pipeline.  Attention processes BOTH local heads together
    per 512-column q-quarter; QKV projection groups and output-projection
    groups are interleaved as fillers into the attention kt-loops.
    """
    g = type("G", (), env)  # attribute access to captured bindings

    x_hi, x_lo = g.x_hi, g.x_lo
    wq_hi, wq_lo, wk_hi, wk_lo = g.wq_hi, g.wq_lo, g.wk_hi, g.wk_lo
    wv_hi, wv_lo, woT, out = g.wv_hi, g.wv_lo, g.woT, g.out
    bq_sb, bo_full = g.bq_sb, g.bo_full
    ones65_sb, warm_src = g.ones65_sb, g.warm_src
    xw_pool, qkv_pool = g.xw_pool, g.qkv_pool
    et_pool, norm_pool, ctxn_pool = g.et_pool, g.norm_pool, g.ctxn_pool
    ow_pool, obuf_pool = g.ow_pool, g.obuf_pool
    ps_mm, ps_log, ps_ctx, dram_pool = g.ps_mm, g.ps_log, g.ps_ctx, g.dram_pool

    # --- weights into SBUF first (k/q weights gate the first projections) ---
    wkh_sb = xw_pool.tile([128, 8, 128], FP8, name="wkh_sb")
    wkl_sb = xw_pool.tile([128, 8, 128], FP8, name="wkl_sb")
    wqh_sb = xw_pool.tile([128, 8, 128], FP8, name="wqh_sb")
    wql_sb = xw_pool.tile([128, 8, 128], FP8, name="wql_sb")
    wvh_sb = xw_pool.tile([128, 8, 130], FP8, name="wvh_sb")
    wvl_sb = xw_pool.tile([128, 8, 130], FP8, name="wvl_sb")

    # --- x^T hi/lo; chunk 0 lands as kt-pair pieces so the first DoubleRow
    #     matmul can start as soon as its pair arrives; hi rides the sync
    #     queue, lo the scalar queue ---
    xh_sb = xw_pool.tile([128, 8, P], FP8, name="xh_sb")  # [part, ktile, pos]
    xl_sb = xw_pool.tile([128, 8, P], FP8, name="xl_sb")
    xh_r = x_hi[:, :].rearrange("(a p) c -> p a c", p=128)
    xl_r = x_lo[:, :].rearrange("(a p) c -> p a c", p=128)
    # NOTHING heavy rides the scalar queue before the attention exps: every
    # HWDGE dispatch there occupies the Activation sequencer and delays the
    # first superiteration's exp chain.  Only x_lo chunk 0 (needed by the
    # very first DoubleRow matmuls) uses scalar; the rest splits between
    # the sync HWDGE queue and the gpsimd SWDGE ring (25ns dispatches).
    # chunk 0 (positions 0:512) lands in DEDICATED tiles so the first
    # projections' DMA-sem waits never coalesce with the later streaming
    # chunks targeting the big x tiles
    xh0_sb = xw_pool.tile([128, 8, 512], FP8, name="xh0_sb")
    xl0_sb = xw_pool.tile([128, 8, 512], FP8, name="xl0_sb")
    # the DMA engines drain transfers in dispatch order, and the first q
    # dr3's waits coalesce over its whole filler -- so EVERY q dependency
    # (wq + full x chunk 0) must be at the very front of both queues; wkh
    # follows (k's dr3 runs after q's anyway)
    nc.scalar.dma_start(out=wqh_sb, in_=wq_hi[:, :].rearrange("(a p) c -> p a c", p=128))
    nc.sync.dma_start(out=xh0_sb[:, 0:4, :], in_=xh_r[:, 0:4, 0:512])
    nc.scalar.dma_start(out=xl0_sb[:, 0:4, :], in_=xl_r[:, 0:4, 0:512])
    nc.sync.dma_start(out=xh0_sb[:, 4:8, :], in_=xh_r[:, 4:8, 0:512])
    nc.scalar.dma_start(out=xl0_sb[:, 4:8, :], in_=xl_r[:, 4:8, 0:512])
    nc.sync.dma_start(out=wkh_sb, in_=wk_hi[:, :].rearrange("(a p) c -> p a c", p=128))
    nc.gpsimd.dma_start(out=wql_sb, in_=wq_lo[:, :].rearrange("(a p) c -> p a c", p=128))
    nc.sync.dma_start(out=wkl_sb, in_=wk_lo[:, :].rearrange("(a p) c -> p a c", p=128))
    nc.sync.dma_start(
        out=wvh_sb, in_=wv_hi[:, :].rearrange("(a p) c -> p a c", p=128)
    )
    nc.gpsimd.dma_start(
        out=wvl_sb, in_=wv_lo[:, :].rearrange("(a p) c -> p a c", p=128)
    )
    for cb in range(1, 8):
        csl = slice(cb * 512, (cb + 1) * 512)
        nc.sync.dma_start(out=xh_sb[:, :, csl], in_=xh_r[:, :, csl])
        eng = nc.gpsimd if cb <= 2 else nc.sync
        eng.dma_start(out=xl_sb[:, :, csl], in_=xl_r[:, :, csl])

    # W_o isn't needed until the first output projection (~half-way in)
    wo_sb = ow_pool.tile([128, 8, D], BF16, name="wo_sb")
    woT_r = woT[:, :].rearrange("(a p) c -> p a c", p=128)
    nc.sync.dma_start(out=wo_sb[:, 0:4, :], in_=woT_r[:, 0:4, :])
    nc.gpsimd.dma_start(out=wo_sb[:, 4:8, :], in_=woT_r[:, 4:8, :])
    nc.gpsimd.dma_start(out=g.bo_full, in_=g.bo[:, :])

    # fp8 hi/lo packs for the DR logits (see module docstring)
    kpk = [qkv_pool.tile([128, P], FP8, name=f"kpk{h}") for h in range(2)]
    qpk = [qkv_pool.tile([128, 2, P], FP8, name=f"qpk{h}") for h in range(2)]
    v_sb = qkv_pool.tile([128, 32, 130], BF16, name="v_sb")
    # softmax-denominator "ones" columns (64 and 129 of every v tile),
    # written once -- the per-tile drains never touch them
    nc.vector.memset(
        v_sb[:, :, :].rearrange("p t (b c) -> p t b c", b=2)[:, :, :, 64], 1.0
    )

    # Per-row-group A2A buffers: group m = rows m*1024 + c*128 .. +128.
    cc_in = [
        dram_pool.tile([NCORES * 128, 128], BF16, name=f"cc_in{m}") for m in range(4)
    ]
    tmp = [
        dram_pool.tile([NCORES * 128, 128], BF16, name=f"tmp{m}") for m in range(4)
    ]
    lw_all = [None] * 4

    # ---- filler units: one PSUM group each, emitted inside attention ----
    def dr3(ps, wh, wl, xh, xl, sl, last_stop):
        """12 DoubleRow matmuls: hi*hi + hi*lo + lo*hi over 4 kt-pairs."""
        terms = [(wh, xh), (wh, xl), (wl, xh)]
        for ti, (w, x) in enumerate(terms):
            for tp in range(4):
                kp = slice(2 * tp, 2 * tp + 2)
                nc.tensor.matmul(
                    out=ps,
                    lhsT=w[:, kp, :],
                    rhs=x[:, kp, sl],
                    start=(ti == 0 and tp == 0),
                    stop=(last_stop and ti == 2 and tp == 3),
                    perf_mode=DR,
                )

    def f_k(p8):
        def emit():
            sl = slice(p8 * 512, (p8 + 1) * 512)
            xh, xl = (xh0_sb, xl0_sb) if p8 == 0 else (xh_sb, xl_sb)
            ps = ps_mm.tile([128, 512], F32, tag="mm", name="ps_k")
            dr3(ps, wkh_sb, wkl_sb, xh, xl, sl, last_stop=True)
            with nc.allow_low_precision(reason="k fp8 hi/lo pack"):
                # h0 pack = [hi; lo], h1 pack = [lo; hi] (order irrelevant
                # inside the contraction) -- keeps every op's in/out offsets
                # in the probe-verified patterns.  Slice 0 is the startup
                # critical path: the hi copies run on the (still idle)
                # Activation engine so the DVE chain halves.
                hi_eng = nc.scalar.copy if p8 == 0 else nc.vector.tensor_copy
                hi_eng(out=kpk[0][0:64, sl], in_=ps[0:64, :])
                nc.vector.tensor_tensor(
                    out=kpk[0][64:128, sl], in0=ps[0:64, :],
                    in1=kpk[0][0:64, sl], op=SUB,
                )
                hi_eng(out=kpk[1][64:128, sl], in_=ps[64:128, :])
                nc.vector.tensor_tensor(
                    out=kpk[1][0:64, sl], in0=ps[64:128, :],
                    in1=kpk[1][64:128, sl], op=SUB,
                )
        return emit

    def f_q(p8):
        def emit():
            sl = slice(p8 * 512, (p8 + 1) * 512)
            xh, xl = (xh0_sb, xl0_sb) if p8 == 0 else (xh_sb, xl_sb)
            ps = ps_mm.tile([128, 512], F32, tag="mm", name="ps_q")
            dr3(ps, wqh_sb, wql_sb, xh, xl, sl, last_stop=True)
            with nc.allow_low_precision(reason="q fp8 hi/lo pack + bias"):
                # native halves: h0 at partitions 0:64, h1 at 64:128; each
                # head's gpsimd duplicate is emitted right after its DVE
                # pair so the Pool copy overlaps the other head's DVE work.
                # Slice 0: the hi+bias ops run on the idle Activation
                # engine (Identity activation with per-partition bias).
                def q_hi(dst, src, bias):
                    if p8 == 0:
                        nc.scalar.add(out=dst, in_=src, add=bias)
                    else:
                        nc.vector.tensor_scalar(
                            out=dst, in0=src, scalar1=bias, scalar2=None,
                            op0=mybir.AluOpType.add,
                        )
                q_hi(qpk[0][0:64, 0, sl], ps[0:64, :], bq_sb[0:64, :])
                nc.vector.scalar_tensor_tensor(
                    out=qpk[0][0:64, 1, sl], in0=ps[0:64, :],
                    scalar=bq_sb[0:64, :], in1=qpk[0][0:64, 0, sl],
                    op0=mybir.AluOpType.add, op1=SUB,
                )
                nc.gpsimd.tensor_copy(
                    out=qpk[0][64:128, :, sl], in_=qpk[0][0:64, :, sl]
                )
                q_hi(qpk[1][64:128, 0, sl], ps[64:128, :], bq_sb[64:128, :])
                nc.vector.scalar_tensor_tensor(
                    out=qpk[1][64:128, 1, sl], in0=ps[64:128, :],
                    scalar=bq_sb[64:128, :], in1=qpk[1][64:128, 0, sl],
                    op0=mybir.AluOpType.add, op1=SUB,
                )
                nc.gpsimd.tensor_copy(
                    out=qpk[1][0:64, :, sl], in_=qpk[1][64:128, :, sl]
                )
        return emit

    def f_v(pt):
        def emit():
            psl = slice(pt * 128, (pt + 1) * 128)
            xh, xl = (xh0_sb, xl0_sb) if pt < 4 else (xh_sb, xl_sb)
            ps = ps_mm.tile([128, 130], F32, tag="mm", name="ps_v")
            terms = [(xh, wvh_sb), (xh, wvl_sb), (xl, wvh_sb)]
            for ti, (x, w) in enumerate(terms):
                for tp in range(4):
                    kp = slice(2 * tp, 2 * tp + 2)
                    nc.tensor.matmul(
                        out=ps,
                        lhsT=x[:, kp, psl],
                        rhs=w[:, kp, :],
                        start=(ti == 0 and tp == 0),
                        stop=(ti == 2 and tp == 3),
                        perf_mode=DR,
                    )
            # drain skips the denominator columns 64/129 (memset to 1.0 once
            # at startup; the fp8 weight columns there are zero-padded)
            with nc.allow_low_precision(reason="v copy with 1/64 descale"):
                nc.vector.tensor_scalar_mul(
                    out=v_sb[:, pt, :].rearrange("p (b c) -> p b c", b=2)[:, :, 0:64],
                    in0=ps[:, :].rearrange("p (b c) -> p b c", b=2)[:, :, 0:64],
                    scalar1=1.0 / WSCALE,
                )
        return emit

    def f_lw(m, fast=False):
        def emit():
            t = obuf_pool.tile([128, 8, 128], BF16, tag=f"lw{m}", name="lw")
            src = tmp[m][:, :].rearrange("(a p) r -> p a r", p=128)
            if fast:
                # tail-critical load: one big piece per HWDGE queue -- the
                # serialized dispatch overhead dominates, so fewer pieces
                # land everything sooner
                nc.sync.dma_start(out=t[:, 0:4, :], in_=src[:, 0:4, :])
                nc.scalar.dma_start(out=t[:, 4:8, :], in_=src[:, 4:8, :])
            else:
                nc.gpsimd.dma_start(out=t[:, 0:4, :], in_=src[:, 0:4, :])
                nc.gpsimd.dma_start(out=t[:, 4:8, :], in_=src[:, 4:8, :])
            lw_all[m] = t
        return emit

    def f_op(m, nt, split_out=False, hot=False):
        def emit():
            lw = lw_all[m]
            ps = ps_mm.tile([128, 512], F32, tag="mm", name="ps_o")
            for kt in range(8):
                nc.tensor.matmul(
                    out=ps, lhsT=lw[:, kt, :],
                    rhs=wo_sb[:, kt, nt * 512 : (nt + 1) * 512],
                    start=(kt == 0), stop=(kt == 7),
                )
                if hot and kt in (0, 2, 4):
                    # keep-alive between data-gated kt pieces: prevents the
                    # p-state from resetting while the lw DMA lands
                    warm(1)
            o_sb = obuf_pool.tile([128, 512], F32, tag="ob", name="o_sb")
            nsl = slice(nt * 512, (nt + 1) * 512)
            if split_out:
                for h, eng in enumerate([nc.sync, nc.scalar]):
                    sl = slice(h * 256, (h + 1) * 256)
                    nc.vector.scalar_tensor_tensor(
                        out=o_sb[:, sl], in0=ps[:, sl], scalar=1.0,
                        in1=bo_full[:, nt * 512 + h * 256 : nt * 512 + (h + 1) * 256],
                        op0=mybir.AluOpType.mult, op1=mybir.AluOpType.add,
                    )
                    eng.dma_start(
                        out=out[m * 128 : (m + 1) * 128,
                                nt * 512 + h * 256 : nt * 512 + (h + 1) * 256],
                        in_=o_sb[:, sl],
                    )
            else:
                nc.vector.scalar_tensor_tensor(
                    out=o_sb, in0=ps, scalar=1.0, in1=bo_full[:, nsl],
                    op0=mybir.AluOpType.mult, op1=mybir.AluOpType.add,
                )
                nc.sync.dma_start(
                    out=out[m * 128 : (m + 1) * 128, nsl], in_=o_sb
                )
        return emit

    def emit_a2a(m):
        nc.gpsimd.collective_compute(
            "AllToAll",
            mybir.AluOpType.bypass,
            replica_groups=[list(range(NCORES))],
            ins=[cc_in[m][:].opt()],
            outs=[tmp[m][:].opt()],
        )

    # pending ctx matmul state: each superiteration's 16 ctx accumulations
    # are deferred into the NEXT superiteration, flushed 2 key-tiles per kt
    # over its first 8 kts -- the PE never stalls on an exp completion
    # semaphore, and the Activation engine's exp chain is never starved by
    # a bulk flush at the si boundary.
    pend = []

    def flush_ctx(budget=None, si=None, only_si=None):
        """Emit pending ctx matmuls.  budget caps the count; si restricts
        to entries belonging to superiterations OTHER than si (i.e. only
        the previous si's backlog); only_si flushes just that si's
        entries."""
        n = 0
        while pend and (budget is None or n < budget):
            psi, kt, ps_c, et = pend[0]
            if si is not None and psi == si:
                break
            if only_si is not None and psi != only_si:
                break
            b = psi[0]
            for hh in range(2):
                nc.tensor.matmul(
                    out=ps_c[hh],
                    lhsT=v_sb[:, b * 16 + kt, 65 * hh : 65 * hh + 65],
                    rhs=et[:, hh * 512 : (hh + 1) * 512],
                    start=(kt == 0),
                    stop=(kt == 15),
                )
            pend.pop(0)
            n += 1

    def emit_attn_part(b, qq, ps_c, kts, fillers=(), slotted=None,
                       flush_budget=2, self_lag=None):
        """Key tiles kts of one superiteration (both heads, q cols qq*512..).
        slotted: {kt: [fn, ...]} fillers pinned to an exact key tile (norm
        chain + collective dispatch); the rest spread evenly.  flush_budget:
        prev-si ctx pairs flushed per kt.  self_lag: also flush THIS si's
        ctx with the given kt lag (latency-critical last superiteration)."""
        fillers = list(fillers)
        slotted = slotted or {}
        co = b * T
        qco = co + qq * 512
        nf = 0
        nkt = len(kts)
        for ki, kt in enumerate(kts):
            ps_l = ps_log.tile([128, 1024], F32, tag="log", name="ps_l")
            for hh in range(2):
                lhsT = kpk[hh][:, co + kt * 128 : co + (kt + 1) * 128]
                nc.tensor.matmul(
                    out=ps_l[:, hh * 512 : (hh + 1) * 512],
                    lhsT=lhsT.unsqueeze(1).broadcast_to([128, 2, 128]),
                    rhs=qpk[hh][:, :, qco : qco + 512],
                    start=True,
                    stop=True,
                    perf_mode=DR,
                )
            flush_ctx(budget=flush_budget, si=(b, qq))
            if self_lag is not None and ki >= self_lag:
                flush_ctx(budget=1)
            for fn in slotted.get(kt, ()):
                fn()
            want = (ki + 1) * len(fillers) // nkt
            while nf < want:
                fillers[nf]()
                nf += 1
            et = et_pool.tile([128, 1024], BF16, tag="et", name="et")
            nc.scalar.activation(
                out=et, in_=ps_l,
                func=mybir.ActivationFunctionType.Exp,
                scale=EXP_SCALE,
            )
            pend.append(((b, qq), kt, ps_c, et))

    def emit_norm_copies(b, qq, ps_c):
        """DVE-only PSUM drain at the end of a superiteration (releases the
        ctx psum banks on the baseline schedule)."""
        flush_ctx(only_si=(b, qq))
        ctxr = norm_pool.tile([65, 1024], F32, tag="ctxr", name="ctxr")
        for hh in range(2):
            nc.vector.tensor_copy(
                out=ctxr[:, hh * 512 : (hh + 1) * 512], in_=ps_c[hh]
            )
        return ctxr

    def f_norm(b, qq, ctxr):
        """Deferred normalize, split in two fillers: [0] reciprocal (DVE
        only), [1] PE broadcast + mul + scatter.  Placing them a few key
        tiles apart in the next superiteration keeps the PE stream from
        ever waiting on the reciprocal."""
        m = 2 * b + qq // 2
        half = qq % 2
        rs = norm_pool.tile([65, 1024], BF16, tag="rsum", name="rs")

        def emit_recip():
            with nc.allow_low_precision(reason="softmax denom bf16 bcast"):
                nc.vector.reciprocal(out=rs[64:65, :], in_=ctxr[64:65, :])

        def emit_mul():
            ctxn = ctxn_pool.tile([64, 1024], BF16, tag="cn", name="ctxn")
            for hh in range(2):
                bc = ps_mm.tile([64, 512], F32, tag="mm", name="bc")
                nc.tensor.matmul(
                    out=bc,
                    lhsT=ones65_sb[64:65, 0:64],
                    rhs=rs[64:65, hh * 512 : (hh + 1) * 512],
                    start=True,
                    stop=True,
                )
                nc.vector.tensor_mul(
                    out=ctxn[:, hh * 512 : (hh + 1) * 512],
                    in0=ctxr[0:64, hh * 512 : (hh + 1) * 512],
                    in1=bc,
                )
                nc.sync.dma_start(
                    out=cc_in[m][:, :].rearrange("(j q) r -> q j r", q=128)[
                        DK * hh : DK * hh + DK, half * 4 : half * 4 + 4, :
                    ],
                    in_=ctxn[:, hh * 512 : (hh + 1) * 512].rearrange(
                        "f (j r) -> f j r", j=4
                    ),
                )
        return emit_recip, emit_mul

    def emit_norm_inline(b, qq, ps_c, prewarm=2):
        """Latency-critical norm (the very last superiteration): reciprocals
        read the colsum rows straight from PSUM, with PE keep-alive matmuls
        covering their latency."""
        flush_ctx(only_si=(b, qq))
        m = 2 * b + qq // 2
        half = qq % 2
        warm(prewarm)
        rs = norm_pool.tile([65, 1024], BF16, tag="rsum", name="rs")
        with nc.allow_low_precision(reason="softmax denom bf16 broadcast"):
            for hh in range(2):
                nc.vector.reciprocal(
                    out=rs[64:65, hh * 512 : (hh + 1) * 512],
                    in_=ps_c[hh][64:65, :],
                )
        ctxr = norm_pool.tile([65, 1024], F32, tag="ctxr", name="ctxr")
        for hh in range(2):
            nc.vector.tensor_copy(
                out=ctxr[:, hh * 512 : (hh + 1) * 512], in_=ps_c[hh]
            )
        ctxn = ctxn_pool.tile([64, 1024], BF16, tag="cn", name="ctxn")
        for hh in range(2):
            bc = ps_mm.tile([64, 512], F32, tag="mm", name="bc")
            nc.tensor.matmul(
                out=bc,
                lhsT=ones65_sb[64:65, 0:64],
                rhs=rs[64:65, hh * 512 : (hh + 1) * 512],
                start=True,
                stop=True,
            )
            nc.vector.tensor_mul(
                out=ctxn[:, hh * 512 : (hh + 1) * 512],
                in0=ctxr[0:64, hh * 512 : (hh + 1) * 512],
                in1=bc,
            )
            nc.sync.dma_start(
                out=cc_in[m][:, :].rearrange("(j q) r -> q j r", q=128)[
                    DK * hh : DK * hh + DK, half * 4 : half * 4 + 4, :
                ],
                in_=ctxn[:, hh * 512 : (hh + 1) * 512].rearrange(
                    "f (j r) -> f j r", j=4
                ),
            )

    def alloc_ps_c():
        return [
            ps_ctx.tile([65, 512], F32, tag="ctx", name=f"psc{hh}")
            for hh in range(2)
        ]

    def emit_attn(b, qq, fillers=(), slotted=None, flush_budget=2):
        """One full superiteration: both heads, q columns qq*512..+512.
        Returns three deferred fillers (psum-drain, recip, broadcast+mul+
        scatter) slotted into the NEXT superiteration at kts 8/11/14, so
        the si boundary never stalls the PE on the exp/normalize chain and
        the DVE chain always has a few kts of slack between stages."""
        ps_c = alloc_ps_c()
        emit_attn_part(b, qq, ps_c, range(16), fillers, slotted,
                       flush_budget=flush_budget)
        ctxr = norm_pool.tile([65, 1024], F32, tag="ctxr", name="ctxr")

        def copies():
            flush_ctx(only_si=(b, qq))
            for hh in range(2):
                nc.vector.tensor_copy(
                    out=ctxr[:, hh * 512 : (hh + 1) * 512], in_=ps_c[hh]
                )
        r, mu = f_norm(b, qq, ctxr)
        return copies, r, mu

    def norm_slots(c, r, mu, extra15=None, fq=None):
        s = {8: [c], 11: [r], 14: [mu]}
        if fq:
            s[2] = [fq]
        if extra15:
            s[15] = list(extra15)
        return s

    def warm(n):
        for _ in range(n):
            ps_d = ps_log.tile([128, 512], F32, tag="log", name="ps_warm")
            nc.tensor.matmul(
                out=ps_d, lhsT=warm_src[:, 0:128], rhs=warm_src,
                start=True, stop=True,
            )

    # ---- emission schedule ----
    # superiteration (0,0) is streamed in 4-kt blocks: each block's k slice
    # and v tiles are emitted (top level) just before the part that consumes
    # them, so attention starts as soon as x chunk 0 lands.  Output
    # projections trail their group's AllToAll by two superiterations (the
    # collective takes ~1.4 superiterations; anything earlier stalls the
    # in-order PE queue on the lw load).
    # q first: its pack chain (4 DVE ops + 2 gpsimd duplicates) is the
    # longest startup dependency; emitting it ahead lets it overlap the k
    # projection's PE work.
    f_q(0)()
    f_k(0)()
    for pt in range(4):
        f_v(pt)()
    ps_c00 = alloc_ps_c()
    for blk in range(4):
        if blk < 3:
            f_k(blk + 1)()
            for pt in range(4 * blk + 4, 4 * blk + 8):
                f_v(pt)()
        emit_attn_part(0, 0, ps_c00, range(4 * blk, 4 * blk + 4),
                       [f_q(1)] if blk == 3 else [])
    ctxr00 = emit_norm_copies(0, 0, ps_c00)
    r00, m00 = f_norm(0, 0, ctxr00)
    c01, r01, m01 = emit_attn(0, 1, [f_k(4), f_k(5), f_k(6), f_k(7)],
                              slotted={2: [f_q(2)], 4: [r00], 8: [m00]})
    c02, r02, m02 = emit_attn(0, 2, [f_v(pt) for pt in range(16, 24)],
                              slotted=norm_slots(c01, r01, m01,
                                                 [lambda: emit_a2a(0)],
                                                 fq=f_q(3)))
    c03, r03, m03 = emit_attn(0, 3, [f_v(pt) for pt in range(24, 32)]
                              + [f_lw(0)],
                              slotted=norm_slots(c02, r02, m02, fq=f_q(4)))
    c10, r10, m10 = emit_attn(1, 0, [],
                              slotted=norm_slots(c03, r03, m03,
                                                 [lambda: emit_a2a(1)],
                                                 fq=f_q(5)))
    c11, r11, m11 = emit_attn(1, 1, [f_lw(1)],
                              slotted=norm_slots(c10, r10, m10, fq=f_q(6)))
    # tail-critical stretch: flush the previous si's ctx at 4 pairs/kt so
    # the norm chains (and with them the last two collectives) fire as
    # early as the data allows
    c12, r12, m12 = emit_attn(1, 2, [],
                              slotted={2: [f_q(7)], 4: [c11], 6: [r11],
                                       8: [m11], 9: [lambda: emit_a2a(2)]},
                              flush_budget=4)
    ps_c13 = alloc_ps_c()
    emit_attn_part(1, 3, ps_c13, range(16), [f_lw(2)],
                   slotted={4: [c12], 6: [r12], 8: [m12]},
                   flush_budget=4, self_lag=1)
    emit_norm_inline(1, 3, ps_c13, prewarm=2)
    emit_a2a(3)
    # the last collective's window hosts ALL trailing output projections
    # (their lw loads completed long before); the leftover is bridged with
    # warm matmuls so the final projection runs at full p-state
    warm(3)  # bridges until lw2 lands (gated by the m2 collective)
    f_op(2, 0)()
    f_op(2, 1)()
    f_op(0, 0)()
    f_op(0, 1)()
    f_op(1, 0)()
    f_op(1, 1)()
    # bridge the whole remaining collective window at full p-state: the
    # final projections start the instant lw3 lands, on a hot PE
    # bridge PAST the point where every lw3 piece has landed (~4us after
    # the collective): the final projections then run gapless at full
    # p-state instead of stuttering at piece boundaries at half rate
    warm(97)
    f_lw(3, fast=True)()
    f_op(3, 0, split_out=True)()
    f_op(3, 1, split_out=True)()


def _fp8_split(a):
    """a (f32) -> (hi, lo) fp8e4m3 with hi + lo ~= a."""
    hi = a.astype(NPFP8)
    lo = (a - hi.astype(np.float32)).astype(NPFP8)
    return hi, lo


def make_in_maps(x, W_qkv, b_qkv, W_o, b_o):
    x = np.asarray(x, dtype=np.float32)
    W_qkv = np.asarray(W_qkv, dtype=np.float32)
    b_qkv = np.asarray(b_qkv, dtype=np.float32)
    W_o = np.asarray(W_o, dtype=np.float32)
    b_o = np.asarray(b_o, dtype=np.float32)

    xT = np.ascontiguousarray(x.reshape(P, D).T)
    x_hi, x_lo = _fp8_split(xT)
    woT = np.ascontiguousarray(W_o.T).astype(NPBF16)
    # fold b_v into the output bias: out += W_o @ b_v
    bv_full = b_qkv[2 * D : 3 * D]
    bo_eff = np.broadcast_to(
        (b_o + W_o @ bv_full).reshape(1, D), (128, D)
    ).astype(NPBF16)

    in_maps = []
    for c in range(NCORES):
        wq = W_qkv[128 * c : 128 * c + 128]  # [128, 1024] q features
        wk = W_qkv[D + 128 * c : D + 128 * c + 128]
        wv = W_qkv[2 * D + 128 * c : 2 * D + 128 * c + 128]
        wvT_pad = np.zeros((D, 130), dtype=np.float32)
        wvT_pad[:, 0:64] = wv[0:64].T
        wvT_pad[:, 65:129] = wv[64:128].T
        # the denominator "ones" columns are memset on-chip; v's real bias
        # was folded into bo_eff (cols 64/129 of the weights stay zero).
        wq_hi, wq_lo = _fp8_split(WSCALE * np.ascontiguousarray(wq.T))
        wk_hi, wk_lo = _fp8_split(WSCALE * np.ascontiguousarray(wk.T))
        wv_hi, wv_lo = _fp8_split(WSCALE * wvT_pad)
        in_maps.append(
            {
                "x_hi": x_hi,
                "x_lo": x_lo,
                "wq_hi": wq_hi,
                "wq_lo": wq_lo,
                "wk_hi": wk_hi,
                "wk_lo": wk_lo,
                "wv_hi": wv_hi,
                "wv_lo": wv_lo,
                "bq": (WSCALE * b_qkv[128 * c : 128 * c + 128])
                .reshape(128, 1)
                .astype(np.float32),
                "woT": woT,
                "bo": bo_eff,
            }
        )
    return in_maps


def assemble_out(outs):
    """outs[c] is [512, 1024]: row tile rt holds global rows
    rt*1024 + c*128 .. +128 (interleaved ownership)."""
    full = np.zeros((P, D), dtype=np.float32)
    for c in range(NCORES):
        oc = np.asarray(outs[c], dtype=np.float32)
        for rt in range(4):
            full[rt * 1024 + c * 128 : rt * 1024 + c * 128 + 128] = oc[
                rt * 128 : (rt + 1) * 128
            ]
    return full.reshape(B, T, D)


_CACHED_GRAPH = None


def kernel(x, W_qkv, b_qkv, W_o, b_o):
    global _CACHED_GRAPH
    if _CACHED_GRAPH is None:
        _CACHED_GRAPH = build_graph()
    nc = _CACHED_GRAPH
    in_maps = make_in_maps(x, W_qkv, b_qkv, W_o, b_o)
    res = run_bass_kernel_spmd(nc, in_maps, core_ids=list(range(NCORES)))
    outs = [res.results[c]["out"] for c in range(NCORES)]
    return assemble_out(outs)
